# revision 32
# baseline (speedup 1.0000x reference)
"""Causal self-attention with RoPE — Trainium2 Bass kernel (v2, pipelined).

Problem: B=8, T=1024, C=768, H=12, D=64; y = proj(softmax(causal(rope(q)·rope(k)))·v)

Sharding: data-parallel over batch — core b computes batch element b end-to-end.

v2 schedule: attention is interleaved with the QKV projection at head-pair
granularity so the Act engine (exp) streams continuously under the PE-critical
path instead of phase-serializing:

  v tiles -> for pair p: [qk mm+rope | scores 2p,2p+1 | PV 2p-2,2p-1 | norm]
  -> proj

Engine balance per pair: PE qk/pswap/scores/PV/bcv-matmuls; Act qk psum->sbuf
copies + exp; DVE rope muls, diag masks (merged per head), ytmp copies, recip,
norm muls, v copies; Pool rope adds + memsets; DMA inputs/sums rows/outputs.
"""

import sys

sys.path.insert(0, "/opt/trn_rl_repo")

import numpy as np
import ml_dtypes

BF16 = ml_dtypes.bfloat16

B, T, C, H = 8, 1024, 768, 12
D = C // H  # 64
NT = T // 128  # 8 t-tiles
NCT = C // 128  # 6 c-tiles
NP = H // 2  # 6 head pairs

_CACHE = {}


def _host_tables():
    inv_freq = 1.0 / (10000.0 ** (np.arange(0, D, 2, dtype=np.float64) / D))  # [32]
    freqs = np.outer(np.arange(T, dtype=np.float64), inv_freq)  # [T, 32]
    cos = np.cos(freqs).astype(np.float32)
    sin = np.sin(freqs).astype(np.float32)
    cos_t = cos.T  # [32, T]
    sin_t = sin.T
    cc = np.concatenate([cos_t, cos_t, cos_t, cos_t], axis=0)  # [128, T]
    ss = np.concatenate([sin_t, -sin_t, sin_t, -sin_t], axis=0)  # [128, T]
    # Pswap (symmetric): within each 64-block swap halves; lhsT = Pswap
    blk = np.zeros((64, 64), np.float32)
    blk[:32, 32:] = np.eye(32)
    blk[32:, :32] = np.eye(32)
    pswap = np.zeros((128, 128), np.float32)
    pswap[:64, :64] = blk
    pswap[64:, 64:] = blk
    # causal keep-mask for diagonal blocks, replicated 8x along free dim
    m01 = (np.arange(128)[:, None] <= np.arange(128)[None, :]).astype(np.float32)
    m01r = np.tile(m01, (1, 8))
    # pair-broadcast selector: psel[a, j] = 1 if j//64 == a
    psel = np.zeros((2, 128), np.float32)
    psel[0, 0:64] = 1.0
    psel[1, 64:128] = 1.0
    return cc, ss, pswap, m01r, psel


def _segs(i):
    """Causal t-segments for s-tile i: list of (t0, width), each within one
    512-col psum bank; first 128 local cols of the first seg are diagonal."""
    s0 = i * 128
    out = []
    if s0 < 512:
        out.append((s0, 512 - s0))
        out.append((512, 512))
    else:
        out.append((s0, 1024 - s0))
    return out


def _build_nc(stage=99):
    import bass_rust
    from concourse import bass, mybir, tile

    f32 = mybir.dt.float32
    bf16 = mybir.dt.bfloat16
    EXP = mybir.ActivationFunctionType.Exp

    def split_multiwaits(nc):
        """Walrus compat: at most one sem wait per instruction — hoist extra
        waits onto preceding same-engine NoOps."""
        n = 0
        for f in nc.m.functions:
            for blk in f.blocks:
                new = []
                for inst in blk.instructions:
                    si = inst.sync_info
                    if si is not None and len(si.on_wait) > 1:
                        waits = list(si.on_wait)
                        for w in waits[:-1]:
                            n += 1
                            new.append(
                                mybir.InstNoOp(
                                    name=f"{inst.name}-sw{n}",
                                    engine=inst.engine,
                                    sync_info=bass_rust.SyncInfo(
                                        on_wait=[w], on_update=[]
                                    ),
                                )
                            )
                        inst.sync_info = bass_rust.SyncInfo(
                            on_wait=[waits[-1]], on_update=list(si.on_update)
                        )
                    new.append(inst)
                blk.instructions = new

    nc = bass.Bass()
    xt_d = nc.declare_dram_parameter("xt", [C, T], bf16, isOutput=False)
    wq_d = nc.declare_dram_parameter("wqkvt", [C, 3 * C], bf16, isOutput=False)
    wp_d = nc.declare_dram_parameter("wprojt", [C, C], bf16, isOutput=False)
    cc_d = nc.declare_dram_parameter("cc", [128, T], bf16, isOutput=False)
    ss_d = nc.declare_dram_parameter("ss", [128, T], bf16, isOutput=False)
    psw_d = nc.declare_dram_parameter("pswap", [128, 128], bf16, isOutput=False)
    m01_d = nc.declare_dram_parameter("m01", [128, 8 * 128], bf16, isOutput=False)
    psel_d = nc.declare_dram_parameter("psel", [2, 128], bf16, isOutput=False)
    y_d = nc.declare_dram_parameter("y", [T, C], f32, isOutput=True)

    with tile.TileContext(nc) as tc:
        with (
            tc.tile_pool(name="persist", bufs=1) as persist,
            tc.tile_pool(name="tmp", bufs=4) as tmp,
            tc.tile_pool(name="ppool", bufs=5) as ppool,
            tc.tile_pool(name="outp", bufs=2) as outp,
            tc.tile_pool(name="pssc", bufs=2, space="PSUM") as pssc,
            tc.tile_pool(name="psyt", bufs=2, space="PSUM") as psyt,
        ):
            # ---- persistent SBUF residents + input DMA (qk weight cols first
            # so the first head pair can start ASAP) ----
            wq_sb = [persist.tile([128, 3 * C], bf16, tag=f"wq{i}", name=f"wq{i}") for i in range(NCT)]
            xt_sb = [persist.tile([128, T], bf16, tag=f"xt{i}", name=f"xt{i}") for i in range(NCT)]
            wp_sb = [persist.tile([128, C], bf16, tag=f"wp{i}", name=f"wp{i}") for i in range(NCT)]
            cc_sb = persist.tile([128, T], bf16, tag="cc")
            ss_sb = persist.tile([128, T], bf16, tag="ss")
            psw_sb = persist.tile([128, 128], bf16, tag="psw")
            m01_sb = persist.tile([128, 8 * 128], bf16, tag="m01")
            psel_sb = persist.tile([2, 128], bf16, tag="psel")
            # qk-projection inputs first (so the exp stream starts ASAP),
            # then rope tables, v weight cols, proj weights
            nc.sync.dma_start(psw_sb[:], psw_d[:])
            for i in range(NCT):
                r0 = i * 128
                nc.sync.dma_start(wq_sb[i][:, 0 : 2 * C], wq_d[r0 : r0 + 128, 0 : 2 * C])
                nc.sync.dma_start(xt_sb[i][:], xt_d[r0 : r0 + 128, :])
            nc.sync.dma_start(cc_sb[:], cc_d[:])
            nc.sync.dma_start(ss_sb[:], ss_d[:])
            nc.sync.dma_start(m01_sb[:], m01_d[:])
            nc.sync.dma_start(psel_sb[:], psel_d[:])

            qk_sb = [persist.tile([128, T], bf16, tag=f"qk{i}", name=f"qk{i}") for i in range(2 * NCT)]
            v_sb = [persist.tile([128, H, D + 1], bf16, tag=f"v{i}", name=f"v{i}") for i in range(NT)]
            yn_sb = [persist.tile([128, T], bf16, tag=f"yn{i}", name=f"yn{i}") for i in range(NCT)]

            # ---- emitters ----
            def emit_qk_mm(jt):
                """QKV projection matmuls for one qk row-tile + psum->sbuf
                copy (DVE, split in halves so it starts after chain 1)."""
                qkps = psyt.tile([128, T], f32, tag="yt", name="qkps")
                old = tmp.tile([128, T], bf16, tag="old", name="old", bufs=2)
                for tch in range(2):
                    t0 = tch * 512
                    for ct in range(NCT):
                        nc.tensor.matmul(
                            qkps[:, t0 : t0 + 512],
                            lhsT=wq_sb[ct][:, jt * 128 : (jt + 1) * 128],
                            rhs=xt_sb[ct][:, t0 : t0 + 512],
                            start=(ct == 0),
                            stop=(ct == NCT - 1),
                        )
                    nc.vector.tensor_copy(old[:, t0 : t0 + 512], qkps[:, t0 : t0 + 512])
                # rope partition swap via SBUF->SBUF DMA (frees the PE from
                # the pswap matmul)
                swp = tmp.tile([128, T], bf16, tag="swp", name="swp", bufs=2)
                for d0, s0 in ((0, 32), (32, 0), (64, 96), (96, 64)):
                    nc.gpsimd.dma_start(swp[d0 : d0 + 32, :], old[s0 : s0 + 32, :])
                return old, swp

            def emit_qk_rope(jt, oldswp):
                """Rope combine for one qk tile: cc/ss muls, add into qk_sb."""
                old, swp = oldswp
                t2 = tmp.tile([128, T], bf16, tag="t2", name="t2", bufs=2)
                nc.vector.tensor_mul(t2[:], old[:], cc_sb[:])
                t1 = tmp.tile([128, T], bf16, tag="t1", name="t1", bufs=2)
                nc.vector.tensor_mul(t1[:], swp[:], ss_sb[:])
                nc.gpsimd.tensor_add(qk_sb[jt][:], t1[:], t2[:])

            def emit_qk(p):
                o1 = emit_qk_mm(p)
                o2 = emit_qk_mm(NCT + p)
                emit_qk_rope(p, o1)
                emit_qk_rope(NCT + p, o2)

            def emit_v(tt):
                for j0, jw, h0, nh in ((0, 512, 0, 8), (512, 256, 8, 4)):
                    ps = pssc.tile([128, T], f32, tag="sc", name="psv")
                    for ct in range(NCT):
                        nc.tensor.matmul(
                            ps[:, :jw],
                            lhsT=xt_sb[ct][:, tt * 128 : (tt + 1) * 128],
                            rhs=wq_sb[ct][:, 2 * C + j0 : 2 * C + j0 + jw],
                            start=(ct == 0),
                            stop=(ct == NCT - 1),
                        )
                    nc.vector.tensor_copy(
                        v_sb[tt][:, h0 : h0 + nh, 0:D],
                        ps[:, :jw].rearrange("p (h d) -> p h d", h=nh),
                    )
                nc.gpsimd.memset(v_sb[tt][:, :, D : D + 1], 1.0)

            # packed per-head p storage: s-tile i occupies cols
            # [POFF[i], POFF[i] + 1024 - 128 i)
            POFF = [0]
            for i in range(NT):
                POFF.append(POFF[-1] + T - 128 * i)
            PTOT = POFF[-1]  # 4608

            def emit_score_tiles(h, ph, lo, hi):
                """Scores + exp + diag-mask for s-tiles [lo, hi) of head h."""
                qt = qk_sb[h // 2]
                kt = qk_sb[NCT + h // 2]
                po = (h % 2) * D
                for i in range(lo, hi):
                    s0 = i * 128
                    off = POFF[i]
                    lk = kt[po : po + D, s0 : s0 + 128]
                    sc = pssc.tile([128, T], f32, tag="sc", name="sc")
                    for t0, w in _segs(i):
                        nc.tensor.matmul(
                            sc[:, t0 : t0 + w],
                            lhsT=lk,
                            rhs=qt[po : po + D, t0 : t0 + w],
                        )
                    nc.scalar.activation(
                        ph[:, off : off + T - s0], sc[:, s0:T], EXP, scale=0.125
                    )
                    nc.gpsimd.tensor_mul(
                        ph[:, off : off + 128],
                        ph[:, off : off + 128],
                        m01_sb[:, i * 128 : (i + 1) * 128],
                    )

            def emit_pv(h, ph):
                yt = psyt.tile([D + 1, T], f32, tag="yt", name="yt")
                bank_first = [True, True]
                writes = [(i, t0, w) for i in range(NT) for (t0, w) in _segs(i)]
                last_for_bank = {}
                for widx, (i, t0, w) in enumerate(writes):
                    last_for_bank[1 if t0 >= 512 else 0] = widx
                for widx, (i, t0, w) in enumerate(writes):
                    s0 = i * 128
                    off = POFF[i]
                    b = 1 if t0 >= 512 else 0
                    nc.tensor.matmul(
                        yt[:, t0 : t0 + w],
                        lhsT=v_sb[i][:, h : h + 1, :],
                        rhs=ph[:, off + t0 - s0 : off + t0 - s0 + w],
                        start=bank_first[b],
                        stop=(last_for_bank[b] == widx),
                    )
                    bank_first[b] = False
                return yt

            def emit_norm_a(p, yts):
                """Early half of pair-p norm (DVE + DMA): ytmp copies off
                psum, sums-row gather, reciprocal."""
                spair = tmp.tile([2, T], bf16, tag="spair", name="spair", bufs=2)
                ytmps = []
                for k, yt in enumerate(yts):
                    ytmp = tmp.tile([D + 1, T], bf16, tag=f"ytmp{k}", name="ytmp", bufs=2)
                    nc.vector.tensor_copy(ytmp[:], yt[:])
                    ytmps.append(ytmp)
                    nc.sync.dma_start(spair[k : k + 1, :], ytmp[D : D + 1, :])
                invb = tmp.tile([2, T], bf16, tag="invb", name="invb", bufs=2)
                with nc.allow_low_precision(reason="softmax denom recip in bf16"):
                    nc.vector.reciprocal(invb[:], spair[:])
                return ytmps, invb

            def emit_norm_b(p, ytmps, invb, pool_tag="sc"):
                """Late half (PE + DVE): paired broadcast of 1/sums, norm muls
                into yn_sb[p]."""
                bcv = (psyt if pool_tag == "yt" else pssc).tile(
                    [128, T], f32, tag=pool_tag, name="bcv"
                )
                for t0 in (0, 512):
                    nc.tensor.matmul(
                        bcv[:, t0 : t0 + 512], lhsT=psel_sb[:], rhs=invb[:, t0 : t0 + 512]
                    )
                for k in range(2):
                    ro = k * D
                    nc.vector.tensor_mul(
                        yn_sb[p][ro : ro + D, :], ytmps[k][0:D, :], bcv[ro : ro + D, :]
                    )

            # ---- main pipeline ----
            ph_tiles = {}

            def sc_part(h, lo, hi):
                if h >= H:
                    return
                if h not in ph_tiles:
                    ph_tiles[h] = ppool.tile([128, PTOT], bf16, tag="p", name="ph")
                emit_score_tiles(h, ph_tiles[h], lo, hi)

            def emit_ytmp(k, yt):
                """Copy one head's PV output off psum; returns its ytmp."""
                ytmp = tmp.tile([D + 1, T], bf16, tag=f"ytmp{k}", name="ytmp", bufs=2)
                nc.vector.tensor_copy(ytmp[:], yt[:])
                return ytmp

            def emit_recip(p, ytmps):
                spair = tmp.tile([2, T], bf16, tag="spair", name="spair", bufs=2)
                for k in range(2):
                    nc.sync.dma_start(spair[k : k + 1, :], ytmps[k][D : D + 1, :])
                invb = tmp.tile([2, T], bf16, tag="invb", name="invb", bufs=2)
                with nc.allow_low_precision(reason="softmax denom recip in bf16"):
                    nc.vector.reciprocal(invb[:], spair[:])
                return invb

            # preamble: qk pairs 0-2 + scores 0-2, v tiles woven as PE filler
            if stage >= 1:
                emit_qk(0)
                for i in range(NCT):
                    r0 = i * 128
                    nc.sync.dma_start(
                        wq_sb[i][:, 2 * C : 3 * C], wq_d[r0 : r0 + 128, 2 * C : 3 * C]
                    )
                emit_qk(1)
                for i in range(NCT):
                    nc.sync.dma_start(wp_sb[i][:], wp_d[i * 128 : (i + 1) * 128, :])
            if stage >= 2:
                sc_part(0, 0, 4)
                oq1 = emit_qk_mm(2)
                sc_part(0, 4, NT)
                emit_qk_rope(2, oq1)
                oq2 = emit_qk_mm(NCT + 2)
                emit_qk_rope(NCT + 2, oq2)
            if stage >= 1:
                emit_v(0)
                emit_v(1)
            if stage >= 2:
                sc_part(1, 0, 4)
            if stage >= 1:
                emit_v(2)
                emit_v(3)
            if stage >= 2:
                sc_part(1, 4, NT)
            if stage >= 1:
                emit_v(4)
                emit_v(5)
            if stage >= 2:
                sc_part(2, 0, 4)
            if stage >= 1:
                emit_v(6)
                emit_v(7)
            if stage >= 2:
                sc_part(2, 4, NT)
                sc_part(3, 0, NT)

            # proj helpers: partial K-chains (ct 0..4) let the tail overlap
            # the last pair's softmax/norm; finish() adds ct=5 and ships out
            def proj_start(tt, pool_tag):
                ps = (psyt if pool_tag == "yt" else pssc).tile(
                    [128, T], f32, tag=pool_tag, name="pjps"
                )
                for j0, jw in ((0, 512), (512, 256)):
                    for ct in range(NCT - 1):
                        nc.tensor.matmul(
                            ps[:, j0 : j0 + jw],
                            lhsT=yn_sb[ct][:, tt * 128 : (tt + 1) * 128],
                            rhs=wp_sb[ct][:, j0 : j0 + jw],
                            start=(ct == 0),
                            stop=False,
                        )
                return ps

            def proj_finish(tt, ps, ksplit=False):
                ct = NCT - 1
                t0 = tt * 128
                for j0, jw in ((0, 512), (512, 256)):
                    if ksplit:
                        # contract the last pair head-by-head so the first mm
                        # only waits on head 2*ct's norm mul
                        for r in (slice(0, D), slice(D, 128)):
                            nc.tensor.matmul(
                                ps[:, j0 : j0 + jw],
                                lhsT=yn_sb[ct][r, t0 : t0 + 128],
                                rhs=wp_sb[ct][r, j0 : j0 + jw],
                                start=False,
                                stop=(r.start == D),
                            )
                    else:
                        nc.tensor.matmul(
                            ps[:, j0 : j0 + jw],
                            lhsT=yn_sb[ct][:, t0 : t0 + 128],
                            rhs=wp_sb[ct][:, j0 : j0 + jw],
                            start=False,
                            stop=True,
                        )
                osb = outp.tile([128, C], f32, tag="osb")
                nc.scalar.copy(osb[:], ps[:, 0:C])
                nc.gpsimd.dma_start(y_d[t0 : t0 + 128, :], osb[:])

            # steady-state blocks: PVs first (their exps landed 2 blocks ago),
            # then the norm chain early, scores/qk woven to keep Act fed
            if stage >= 3:
                for p in range(NP):
                    last = p == NP - 1
                    yt0 = emit_pv(2 * p, ph_tiles.pop(2 * p))
                    yp0 = emit_ytmp(0, yt0) if stage >= 4 else None
                    yt1 = emit_pv(2 * p + 1, ph_tiles.pop(2 * p + 1))
                    if stage >= 4:
                        yp1 = emit_ytmp(1, yt1)
                        invb = emit_recip(p, (yp0, yp1))
                    sc_part(2 * p + 4, 0, 4)
                    if p + 3 < NP:
                        oq1 = emit_qk_mm(p + 3)
                    if last and stage >= 5:
                        pj0 = proj_start(0, "sc")
                        pj1 = proj_start(1, "sc")
                    sc_part(2 * p + 4, 4, NT)
                    if p + 3 < NP:
                        emit_qk_rope(p + 3, oq1)
                        oq2 = emit_qk_mm(NCT + p + 3)
                    if last and stage >= 5:
                        pj2 = proj_start(2, "yt")
                    sc_part(2 * p + 5, 0, 4)
                    if stage >= 4:
                        emit_norm_b(p, (yp0, yp1), invb, "yt" if last else "sc")
                    if p + 3 < NP:
                        emit_qk_rope(NCT + p + 3, oq2)
                    sc_part(2 * p + 5, 4, NT)
                    if last and stage >= 5:
                        proj_finish(0, pj0, ksplit=True)
                        proj_finish(1, pj1)
                        proj_finish(2, pj2)

            # ---- debug probes for truncated stages ----
            if stage < 5:
                yb = y_d[:].bitcast(bf16)  # [T, 2C] bf16 view of the fp32 output
                if stage == 0:
                    nc.gpsimd.dma_start(yb[0:128, 0:T], xt_sb[0][:])
                elif stage == 1:
                    nc.gpsimd.dma_start(yb[0:128, 0:T], qk_sb[0][:])
                    nc.gpsimd.dma_start(yb[128:256, 0:T], qk_sb[6][:])
                elif stage == 2:
                    nc.gpsimd.dma_start(
                        yb[0:128, 0 : H * (D + 1)],
                        v_sb[0][:].rearrange("p h d -> p (h d)"),
                    )
                elif stage >= 4:
                    nc.gpsimd.dma_start(yb[0:128, 0:T], yn_sb[0][:])

            # ---- proj: out = yT_norm.T @ w_projT (tt 0-2 handled in-block) ----
            for tt in range(3, NT) if stage >= 5 else []:
                ps = proj_start(tt, ("sc", "yt")[tt % 2])
                proj_finish(tt, ps)

    split_multiwaits(nc)
    return nc


def _get_compiled():
    if "nc" not in _CACHE:
        _CACHE["nc"] = _build_nc()
        cc, ss, pswap, m01, psel = _host_tables()
        _CACHE["tables"] = {
            "cc": cc.astype(BF16),
            "ss": ss.astype(BF16),
            "pswap": pswap.astype(BF16),
            "m01": m01.astype(BF16),
            "psel": psel.astype(BF16),
        }
    return _CACHE["nc"], _CACHE["tables"]


def kernel(x, w_qkv, w_proj):
    from concourse.bass_utils import run_bass_kernel_spmd

    nc, tables = _get_compiled()
    x = np.asarray(x, dtype=np.float32)
    wq_t = np.ascontiguousarray(np.asarray(w_qkv, np.float32).T).astype(BF16)
    wp_t = np.ascontiguousarray(np.asarray(w_proj, np.float32).T).astype(BF16)
    in_maps = []
    for b in range(B):
        in_maps.append(
            {
                "xt": np.ascontiguousarray(x[b].T).astype(BF16),
                "wqkvt": wq_t,
                "wprojt": wp_t,
                **tables,
            }
        )
    res = run_bass_kernel_spmd(nc, in_maps, core_ids=list(range(B)))
    return np.stack([res.results[b]["y"].astype(np.float32) for b in range(B)], axis=0)


# revision 33
# speedup vs baseline: 1.2180x; 1.2180x over previous
"""Causal self-attention with RoPE — Trainium2 Bass kernel (v2, pipelined).

Problem: B=8, T=1024, C=768, H=12, D=64; y = proj(softmax(causal(rope(q)·rope(k)))·v)

Sharding: data-parallel over batch — core b computes batch element b end-to-end.

v2 schedule: attention is interleaved with the QKV projection at head-pair
granularity so the Act engine (exp) streams continuously under the PE-critical
path instead of phase-serializing:

  v tiles -> for pair p: [qk mm+rope | scores 2p,2p+1 | PV 2p-2,2p-1 | norm]
  -> proj

Engine balance per pair: PE qk/pswap/scores/PV/bcv-matmuls; Act qk psum->sbuf
copies + exp; DVE rope muls, diag masks (merged per head), ytmp copies, recip,
norm muls, v copies; Pool rope adds + memsets; DMA inputs/sums rows/outputs.
"""

import sys

sys.path.insert(0, "/opt/trn_rl_repo")

import numpy as np
import ml_dtypes

BF16 = ml_dtypes.bfloat16

B, T, C, H = 8, 1024, 768, 12
D = C // H  # 64
NT = T // 128  # 8 t-tiles
NCT = C // 128  # 6 c-tiles
NP = H // 2  # 6 head pairs

_CACHE = {}


def _host_tables():
    inv_freq = 1.0 / (10000.0 ** (np.arange(0, D, 2, dtype=np.float64) / D))  # [32]
    freqs = np.outer(np.arange(T, dtype=np.float64), inv_freq)  # [T, 32]
    cos = np.cos(freqs).astype(np.float32)
    sin = np.sin(freqs).astype(np.float32)
    cos_t = cos.T  # [32, T]
    sin_t = sin.T
    cc = np.concatenate([cos_t, cos_t, cos_t, cos_t], axis=0)  # [128, T]
    ss = np.concatenate([sin_t, -sin_t, sin_t, -sin_t], axis=0)  # [128, T]
    # Pswap (symmetric): within each 64-block swap halves; lhsT = Pswap
    blk = np.zeros((64, 64), np.float32)
    blk[:32, 32:] = np.eye(32)
    blk[32:, :32] = np.eye(32)
    pswap = np.zeros((128, 128), np.float32)
    pswap[:64, :64] = blk
    pswap[64:, 64:] = blk
    # causal keep-mask for diagonal blocks, replicated 8x along free dim
    m01 = (np.arange(128)[:, None] <= np.arange(128)[None, :]).astype(np.float32)
    m01r = np.tile(m01, (1, 8))
    # pair-broadcast selector: psel[a, j] = 1 if j//64 == a
    psel = np.zeros((2, 128), np.float32)
    psel[0, 0:64] = 1.0
    psel[1, 64:128] = 1.0
    return cc, ss, pswap, m01r, psel


def _segs(i):
    """Causal t-segments for s-tile i: list of (t0, width), each within one
    512-col psum bank; first 128 local cols of the first seg are diagonal."""
    s0 = i * 128
    out = []
    if s0 < 512:
        out.append((s0, 512 - s0))
        out.append((512, 512))
    else:
        out.append((s0, 1024 - s0))
    return out


def _build_nc(stage=99):
    import bass_rust
    from concourse import bass, mybir, tile

    f32 = mybir.dt.float32
    bf16 = mybir.dt.bfloat16
    EXP = mybir.ActivationFunctionType.Exp

    def split_multiwaits(nc):
        """Walrus compat: at most one sem wait per instruction — hoist extra
        waits onto preceding same-engine NoOps."""
        n = 0
        for f in nc.m.functions:
            for blk in f.blocks:
                new = []
                for inst in blk.instructions:
                    si = inst.sync_info
                    if si is not None and len(si.on_wait) > 1:
                        waits = list(si.on_wait)
                        for w in waits[:-1]:
                            n += 1
                            new.append(
                                mybir.InstNoOp(
                                    name=f"{inst.name}-sw{n}",
                                    engine=inst.engine,
                                    sync_info=bass_rust.SyncInfo(
                                        on_wait=[w], on_update=[]
                                    ),
                                )
                            )
                        inst.sync_info = bass_rust.SyncInfo(
                            on_wait=[waits[-1]], on_update=list(si.on_update)
                        )
                    new.append(inst)
                blk.instructions = new

    nc = bass.Bass()
    xt_d = nc.declare_dram_parameter("xt", [C, T], bf16, isOutput=False)
    wq_d = nc.declare_dram_parameter("wqkvt", [C, 3 * C], bf16, isOutput=False)
    wp_d = nc.declare_dram_parameter("wprojt", [C, C], bf16, isOutput=False)
    cc_d = nc.declare_dram_parameter("cc", [128, T], bf16, isOutput=False)
    ss_d = nc.declare_dram_parameter("ss", [128, T], bf16, isOutput=False)
    psw_d = nc.declare_dram_parameter("pswap", [128, 128], bf16, isOutput=False)
    m01_d = nc.declare_dram_parameter("m01", [128, 8 * 128], bf16, isOutput=False)
    psel_d = nc.declare_dram_parameter("psel", [2, 128], bf16, isOutput=False)
    y_d = nc.declare_dram_parameter("y", [T, C], f32, isOutput=True)

    with tile.TileContext(nc) as tc:
        with (
            tc.tile_pool(name="persist", bufs=1) as persist,
            tc.tile_pool(name="tmp", bufs=4) as tmp,
            tc.tile_pool(name="ppool", bufs=5) as ppool,
            tc.tile_pool(name="outp", bufs=2) as outp,
            tc.tile_pool(name="pssc", bufs=2, space="PSUM") as pssc,
            tc.tile_pool(name="psyt", bufs=2, space="PSUM") as psyt,
        ):
            # ---- persistent SBUF residents + input DMA (qk weight cols first
            # so the first head pair can start ASAP) ----
            wq_sb = [persist.tile([128, 3 * C], bf16, tag=f"wq{i}", name=f"wq{i}") for i in range(NCT)]
            xt_sb = [persist.tile([128, T], bf16, tag=f"xt{i}", name=f"xt{i}") for i in range(NCT)]
            wp_sb = [persist.tile([128, C], bf16, tag=f"wp{i}", name=f"wp{i}") for i in range(NCT)]
            cc_sb = persist.tile([128, T], bf16, tag="cc")
            ss_sb = persist.tile([128, T], bf16, tag="ss")
            psw_sb = persist.tile([128, 128], bf16, tag="psw")
            m01_sb = persist.tile([128, 8 * 128], bf16, tag="m01")
            psel_sb = persist.tile([2, 128], bf16, tag="psel")
            # qk-projection inputs first (so the exp stream starts ASAP),
            # then rope tables, v weight cols, proj weights
            nc.sync.dma_start(psw_sb[:], psw_d[:])
            for i in range(NCT):
                r0 = i * 128
                nc.sync.dma_start(wq_sb[i][:, 0 : 2 * C], wq_d[r0 : r0 + 128, 0 : 2 * C])
                nc.sync.dma_start(xt_sb[i][:], xt_d[r0 : r0 + 128, :])
            nc.sync.dma_start(cc_sb[:], cc_d[:])
            nc.sync.dma_start(ss_sb[:], ss_d[:])
            nc.sync.dma_start(m01_sb[:], m01_d[:])
            nc.sync.dma_start(psel_sb[:], psel_d[:])

            qk_sb = [persist.tile([128, T], bf16, tag=f"qk{i}", name=f"qk{i}") for i in range(2 * NCT)]
            v_sb = [persist.tile([128, H, D + 1], bf16, tag=f"v{i}", name=f"v{i}") for i in range(NT)]
            yn_sb = [persist.tile([128, T], bf16, tag=f"yn{i}", name=f"yn{i}") for i in range(NCT)]

            # ---- emitters ----
            def emit_qk_mm(jt):
                """QKV projection matmuls for one qk row-tile + psum->sbuf
                copy (DVE, split in halves so it starts after chain 1)."""
                qkps = psyt.tile([128, T], f32, tag="yt", name="qkps")
                old = tmp.tile([128, T], bf16, tag="old", name="old", bufs=2)
                for tch in range(2):
                    t0 = tch * 512
                    for ct in range(NCT):
                        nc.tensor.matmul(
                            qkps[:, t0 : t0 + 512],
                            lhsT=wq_sb[ct][:, jt * 128 : (jt + 1) * 128],
                            rhs=xt_sb[ct][:, t0 : t0 + 512],
                            start=(ct == 0),
                            stop=(ct == NCT - 1),
                        )
                    nc.vector.tensor_copy(old[:, t0 : t0 + 512], qkps[:, t0 : t0 + 512])
                # rope partition swap via SBUF->SBUF DMA (frees the PE from
                # the pswap matmul)
                swp = tmp.tile([128, T], bf16, tag="swp", name="swp", bufs=2)
                for d0, s0 in ((0, 32), (32, 0), (64, 96), (96, 64)):
                    nc.sync.dma_start(swp[d0 : d0 + 32, :], old[s0 : s0 + 32, :])
                return old, swp

            def emit_qk_rope(jt, oldswp):
                """Rope combine for one qk tile: cc/ss muls, add into qk_sb."""
                old, swp = oldswp
                t2 = tmp.tile([128, T], bf16, tag="t2", name="t2", bufs=2)
                nc.vector.tensor_mul(t2[:], old[:], cc_sb[:])
                t1 = tmp.tile([128, T], bf16, tag="t1", name="t1", bufs=2)
                nc.vector.tensor_mul(t1[:], swp[:], ss_sb[:])
                nc.gpsimd.tensor_add(qk_sb[jt][:], t1[:], t2[:])

            def emit_qk(p):
                o1 = emit_qk_mm(p)
                o2 = emit_qk_mm(NCT + p)
                emit_qk_rope(p, o1)
                emit_qk_rope(NCT + p, o2)

            def emit_v(tt):
                for j0, jw, h0, nh in ((0, 512, 0, 8), (512, 256, 8, 4)):
                    ps = pssc.tile([128, T], f32, tag="sc", name="psv")
                    for ct in range(NCT):
                        nc.tensor.matmul(
                            ps[:, :jw],
                            lhsT=xt_sb[ct][:, tt * 128 : (tt + 1) * 128],
                            rhs=wq_sb[ct][:, 2 * C + j0 : 2 * C + j0 + jw],
                            start=(ct == 0),
                            stop=(ct == NCT - 1),
                        )
                    nc.vector.tensor_copy(
                        v_sb[tt][:, h0 : h0 + nh, 0:D],
                        ps[:, :jw].rearrange("p (h d) -> p h d", h=nh),
                    )
                nc.gpsimd.memset(v_sb[tt][:, :, D : D + 1], 1.0)

            # packed per-head p storage: s-tile i occupies cols
            # [POFF[i], POFF[i] + 1024 - 128 i)
            POFF = [0]
            for i in range(NT):
                POFF.append(POFF[-1] + T - 128 * i)
            PTOT = POFF[-1]  # 4608

            def emit_score_tiles(h, ph, lo, hi):
                """Scores + exp + diag-mask for s-tiles [lo, hi) of head h."""
                qt = qk_sb[h // 2]
                kt = qk_sb[NCT + h // 2]
                po = (h % 2) * D
                for i in range(lo, hi):
                    s0 = i * 128
                    off = POFF[i]
                    lk = kt[po : po + D, s0 : s0 + 128]
                    sc = pssc.tile([128, T], f32, tag="sc", name="sc")
                    for t0, w in _segs(i):
                        nc.tensor.matmul(
                            sc[:, t0 : t0 + w],
                            lhsT=lk,
                            rhs=qt[po : po + D, t0 : t0 + w],
                        )
                    nc.scalar.activation(
                        ph[:, off : off + T - s0], sc[:, s0:T], EXP, scale=0.125
                    )
                    nc.gpsimd.tensor_mul(
                        ph[:, off : off + 128],
                        ph[:, off : off + 128],
                        m01_sb[:, i * 128 : (i + 1) * 128],
                    )

            def emit_pv(h, ph):
                yt = psyt.tile([D + 1, T], f32, tag="yt", name="yt")
                bank_first = [True, True]
                writes = [(i, t0, w) for i in range(NT) for (t0, w) in _segs(i)]
                last_for_bank = {}
                for widx, (i, t0, w) in enumerate(writes):
                    last_for_bank[1 if t0 >= 512 else 0] = widx
                for widx, (i, t0, w) in enumerate(writes):
                    s0 = i * 128
                    off = POFF[i]
                    b = 1 if t0 >= 512 else 0
                    nc.tensor.matmul(
                        yt[:, t0 : t0 + w],
                        lhsT=v_sb[i][:, h : h + 1, :],
                        rhs=ph[:, off + t0 - s0 : off + t0 - s0 + w],
                        start=bank_first[b],
                        stop=(last_for_bank[b] == widx),
                    )
                    bank_first[b] = False
                return yt

            def emit_norm_a(p, yts):
                """Early half of pair-p norm (DVE + DMA): ytmp copies off
                psum, sums-row gather, reciprocal."""
                spair = tmp.tile([2, T], bf16, tag="spair", name="spair", bufs=2)
                ytmps = []
                for k, yt in enumerate(yts):
                    ytmp = tmp.tile([D + 1, T], bf16, tag=f"ytmp{k}", name="ytmp", bufs=2)
                    nc.vector.tensor_copy(ytmp[:], yt[:])
                    ytmps.append(ytmp)
                    nc.sync.dma_start(spair[k : k + 1, :], ytmp[D : D + 1, :])
                invb = tmp.tile([2, T], bf16, tag="invb", name="invb", bufs=2)
                with nc.allow_low_precision(reason="softmax denom recip in bf16"):
                    nc.vector.reciprocal(invb[:], spair[:])
                return ytmps, invb

            def emit_norm_b(p, ytmps, invb, pool_tag="sc"):
                """Late half (PE + DVE): paired broadcast of 1/sums, norm muls
                into yn_sb[p]."""
                bcv = (psyt if pool_tag == "yt" else pssc).tile(
                    [128, T], f32, tag=pool_tag, name="bcv"
                )
                for t0 in (0, 512):
                    nc.tensor.matmul(
                        bcv[:, t0 : t0 + 512], lhsT=psel_sb[:], rhs=invb[:, t0 : t0 + 512]
                    )
                for k in range(2):
                    ro = k * D
                    nc.vector.tensor_mul(
                        yn_sb[p][ro : ro + D, :], ytmps[k][0:D, :], bcv[ro : ro + D, :]
                    )

            # ---- main pipeline ----
            ph_tiles = {}

            def sc_part(h, lo, hi):
                if h >= H:
                    return
                if h not in ph_tiles:
                    ph_tiles[h] = ppool.tile([128, PTOT], bf16, tag="p", name="ph")
                emit_score_tiles(h, ph_tiles[h], lo, hi)

            def emit_ytmp(k, yt):
                """Copy one head's PV output off psum; returns its ytmp."""
                ytmp = tmp.tile([D + 1, T], bf16, tag=f"ytmp{k}", name="ytmp", bufs=2)
                nc.vector.tensor_copy(ytmp[:], yt[:])
                return ytmp

            def emit_recip(p, ytmps):
                spair = tmp.tile([2, T], bf16, tag="spair", name="spair", bufs=2)
                for k in range(2):
                    nc.sync.dma_start(spair[k : k + 1, :], ytmps[k][D : D + 1, :])
                invb = tmp.tile([2, T], bf16, tag="invb", name="invb", bufs=2)
                with nc.allow_low_precision(reason="softmax denom recip in bf16"):
                    nc.vector.reciprocal(invb[:], spair[:])
                return invb

            # preamble: qk pairs 0-2 + scores 0-2, v tiles woven as PE filler
            if stage >= 1:
                emit_qk(0)
                for i in range(NCT):
                    r0 = i * 128
                    nc.sync.dma_start(
                        wq_sb[i][:, 2 * C : 3 * C], wq_d[r0 : r0 + 128, 2 * C : 3 * C]
                    )
                emit_qk(1)
                for i in range(NCT):
                    nc.sync.dma_start(wp_sb[i][:], wp_d[i * 128 : (i + 1) * 128, :])
            if stage >= 2:
                sc_part(0, 0, 4)
                oq1 = emit_qk_mm(2)
                sc_part(0, 4, NT)
                emit_qk_rope(2, oq1)
                oq2 = emit_qk_mm(NCT + 2)
                emit_qk_rope(NCT + 2, oq2)
            if stage >= 1:
                emit_v(0)
                emit_v(1)
            if stage >= 2:
                sc_part(1, 0, 4)
            if stage >= 1:
                emit_v(2)
                emit_v(3)
            if stage >= 2:
                sc_part(1, 4, NT)
            if stage >= 1:
                emit_v(4)
                emit_v(5)
            if stage >= 2:
                sc_part(2, 0, 4)
            if stage >= 1:
                emit_v(6)
                emit_v(7)
            if stage >= 2:
                sc_part(2, 4, NT)
                sc_part(3, 0, NT)

            # proj helpers: partial K-chains (ct 0..4) let the tail overlap
            # the last pair's softmax/norm; finish() adds ct=5 and ships out
            def proj_start(tt, pool_tag):
                ps = (psyt if pool_tag == "yt" else pssc).tile(
                    [128, T], f32, tag=pool_tag, name="pjps"
                )
                for j0, jw in ((0, 512), (512, 256)):
                    for ct in range(NCT - 1):
                        nc.tensor.matmul(
                            ps[:, j0 : j0 + jw],
                            lhsT=yn_sb[ct][:, tt * 128 : (tt + 1) * 128],
                            rhs=wp_sb[ct][:, j0 : j0 + jw],
                            start=(ct == 0),
                            stop=False,
                        )
                return ps

            def proj_finish(tt, ps, ksplit=False):
                ct = NCT - 1
                t0 = tt * 128
                for j0, jw in ((0, 512), (512, 256)):
                    if ksplit:
                        # contract the last pair head-by-head so the first mm
                        # only waits on head 2*ct's norm mul
                        for r in (slice(0, D), slice(D, 128)):
                            nc.tensor.matmul(
                                ps[:, j0 : j0 + jw],
                                lhsT=yn_sb[ct][r, t0 : t0 + 128],
                                rhs=wp_sb[ct][r, j0 : j0 + jw],
                                start=False,
                                stop=(r.start == D),
                            )
                    else:
                        nc.tensor.matmul(
                            ps[:, j0 : j0 + jw],
                            lhsT=yn_sb[ct][:, t0 : t0 + 128],
                            rhs=wp_sb[ct][:, j0 : j0 + jw],
                            start=False,
                            stop=True,
                        )
                osb = outp.tile([128, C], f32, tag="osb")
                nc.scalar.copy(osb[:], ps[:, 0:C])
                nc.gpsimd.dma_start(y_d[t0 : t0 + 128, :], osb[:])

            # steady-state blocks: PVs first (their exps landed 2 blocks ago),
            # then the norm chain early, scores/qk woven to keep Act fed
            if stage >= 3:
                for p in range(NP):
                    last = p == NP - 1
                    yt0 = emit_pv(2 * p, ph_tiles.pop(2 * p))
                    yp0 = emit_ytmp(0, yt0) if stage >= 4 else None
                    yt1 = emit_pv(2 * p + 1, ph_tiles.pop(2 * p + 1))
                    if stage >= 4:
                        yp1 = emit_ytmp(1, yt1)
                        invb = emit_recip(p, (yp0, yp1))
                    sc_part(2 * p + 4, 0, 4)
                    if p + 3 < NP:
                        oq1 = emit_qk_mm(p + 3)
                    if last and stage >= 5:
                        pj0 = proj_start(0, "sc")
                        pj1 = proj_start(1, "sc")
                    sc_part(2 * p + 4, 4, NT)
                    if p + 3 < NP:
                        emit_qk_rope(p + 3, oq1)
                        oq2 = emit_qk_mm(NCT + p + 3)
                    if last and stage >= 5:
                        pj2 = proj_start(2, "yt")
                    sc_part(2 * p + 5, 0, 4)
                    if stage >= 4:
                        emit_norm_b(p, (yp0, yp1), invb, "yt" if last else "sc")
                    if p + 3 < NP:
                        emit_qk_rope(NCT + p + 3, oq2)
                    sc_part(2 * p + 5, 4, NT)
                    if last and stage >= 5:
                        proj_finish(0, pj0, ksplit=True)
                        proj_finish(1, pj1)
                        proj_finish(2, pj2)

            # ---- debug probes for truncated stages ----
            if stage < 5:
                yb = y_d[:].bitcast(bf16)  # [T, 2C] bf16 view of the fp32 output
                if stage == 0:
                    nc.gpsimd.dma_start(yb[0:128, 0:T], xt_sb[0][:])
                elif stage == 1:
                    nc.gpsimd.dma_start(yb[0:128, 0:T], qk_sb[0][:])
                    nc.gpsimd.dma_start(yb[128:256, 0:T], qk_sb[6][:])
                elif stage == 2:
                    nc.gpsimd.dma_start(
                        yb[0:128, 0 : H * (D + 1)],
                        v_sb[0][:].rearrange("p h d -> p (h d)"),
                    )
                elif stage >= 4:
                    nc.gpsimd.dma_start(yb[0:128, 0:T], yn_sb[0][:])

            # ---- proj: out = yT_norm.T @ w_projT (tt 0-2 handled in-block) ----
            for tt in range(3, NT) if stage >= 5 else []:
                ps = proj_start(tt, ("sc", "yt")[tt % 2])
                proj_finish(tt, ps)

    split_multiwaits(nc)
    return nc


def _get_compiled():
    if "nc" not in _CACHE:
        _CACHE["nc"] = _build_nc()
        cc, ss, pswap, m01, psel = _host_tables()
        _CACHE["tables"] = {
            "cc": cc.astype(BF16),
            "ss": ss.astype(BF16),
            "pswap": pswap.astype(BF16),
            "m01": m01.astype(BF16),
            "psel": psel.astype(BF16),
        }
    return _CACHE["nc"], _CACHE["tables"]


def kernel(x, w_qkv, w_proj):
    from concourse.bass_utils import run_bass_kernel_spmd

    nc, tables = _get_compiled()
    x = np.asarray(x, dtype=np.float32)
    wq_t = np.ascontiguousarray(np.asarray(w_qkv, np.float32).T).astype(BF16)
    wp_t = np.ascontiguousarray(np.asarray(w_proj, np.float32).T).astype(BF16)
    in_maps = []
    for b in range(B):
        in_maps.append(
            {
                "xt": np.ascontiguousarray(x[b].T).astype(BF16),
                "wqkvt": wq_t,
                "wprojt": wp_t,
                **tables,
            }
        )
    res = run_bass_kernel_spmd(nc, in_maps, core_ids=list(range(B)))
    return np.stack([res.results[b]["y"].astype(np.float32) for b in range(B)], axis=0)


# revision 34
# speedup vs baseline: 1.2571x; 1.0321x over previous
"""Causal self-attention with RoPE — Trainium2 Bass kernel (v2, pipelined).

Problem: B=8, T=1024, C=768, H=12, D=64; y = proj(softmax(causal(rope(q)·rope(k)))·v)

Sharding: data-parallel over batch — core b computes batch element b end-to-end.

v2 schedule: attention is interleaved with the QKV projection at head-pair
granularity so the Act engine (exp) streams continuously under the PE-critical
path instead of phase-serializing:

  v tiles -> for pair p: [qk mm+rope | scores 2p,2p+1 | PV 2p-2,2p-1 | norm]
  -> proj

Engine balance per pair: PE qk/pswap/scores/PV/bcv-matmuls; Act qk psum->sbuf
copies + exp; DVE rope muls, diag masks (merged per head), ytmp copies, recip,
norm muls, v copies; Pool rope adds + memsets; DMA inputs/sums rows/outputs.
"""

import sys

sys.path.insert(0, "/opt/trn_rl_repo")

import numpy as np
import ml_dtypes

BF16 = ml_dtypes.bfloat16

B, T, C, H = 8, 1024, 768, 12
D = C // H  # 64
NT = T // 128  # 8 t-tiles
NCT = C // 128  # 6 c-tiles
NP = H // 2  # 6 head pairs

_CACHE = {}


def _host_tables():
    inv_freq = 1.0 / (10000.0 ** (np.arange(0, D, 2, dtype=np.float64) / D))  # [32]
    freqs = np.outer(np.arange(T, dtype=np.float64), inv_freq)  # [T, 32]
    cos = np.cos(freqs).astype(np.float32)
    sin = np.sin(freqs).astype(np.float32)
    cos_t = cos.T  # [32, T]
    sin_t = sin.T
    cc = np.concatenate([cos_t, cos_t, cos_t, cos_t], axis=0)  # [128, T]
    ss = np.concatenate([sin_t, -sin_t, sin_t, -sin_t], axis=0)  # [128, T]
    # Pswap (symmetric): within each 64-block swap halves; lhsT = Pswap
    blk = np.zeros((64, 64), np.float32)
    blk[:32, 32:] = np.eye(32)
    blk[32:, :32] = np.eye(32)
    pswap = np.zeros((128, 128), np.float32)
    pswap[:64, :64] = blk
    pswap[64:, 64:] = blk
    # causal keep-mask for diagonal blocks, replicated 8x along free dim
    m01 = (np.arange(128)[:, None] <= np.arange(128)[None, :]).astype(np.float32)
    m01r = np.tile(m01, (1, 8))
    # pair-broadcast selector: psel[a, j] = 1 if j//64 == a
    psel = np.zeros((2, 128), np.float32)
    psel[0, 0:64] = 1.0
    psel[1, 64:128] = 1.0
    return cc, ss, pswap, m01r, psel


def _segs(i):
    """Causal t-segments for s-tile i: list of (t0, width), each within one
    512-col psum bank; first 128 local cols of the first seg are diagonal."""
    s0 = i * 128
    out = []
    if s0 < 512:
        out.append((s0, 512 - s0))
        out.append((512, 512))
    else:
        out.append((s0, 1024 - s0))
    return out


def _build_nc(stage=99):
    import bass_rust
    from concourse import bass, mybir, tile

    f32 = mybir.dt.float32
    bf16 = mybir.dt.bfloat16
    EXP = mybir.ActivationFunctionType.Exp

    def split_multiwaits(nc):
        """Walrus compat: at most one sem wait per instruction — hoist extra
        waits onto preceding same-engine NoOps."""
        n = 0
        for f in nc.m.functions:
            for blk in f.blocks:
                new = []
                for inst in blk.instructions:
                    si = inst.sync_info
                    if si is not None and len(si.on_wait) > 1:
                        waits = list(si.on_wait)
                        for w in waits[:-1]:
                            n += 1
                            new.append(
                                mybir.InstNoOp(
                                    name=f"{inst.name}-sw{n}",
                                    engine=inst.engine,
                                    sync_info=bass_rust.SyncInfo(
                                        on_wait=[w], on_update=[]
                                    ),
                                )
                            )
                        inst.sync_info = bass_rust.SyncInfo(
                            on_wait=[waits[-1]], on_update=list(si.on_update)
                        )
                    new.append(inst)
                blk.instructions = new

    nc = bass.Bass()
    xt_d = nc.declare_dram_parameter("xt", [C, T], bf16, isOutput=False)
    wq_d = nc.declare_dram_parameter("wqkvt", [C, 3 * C], bf16, isOutput=False)
    wp_d = nc.declare_dram_parameter("wprojt", [C, C], bf16, isOutput=False)
    cc_d = nc.declare_dram_parameter("cc", [128, T], bf16, isOutput=False)
    ss_d = nc.declare_dram_parameter("ss", [128, T], bf16, isOutput=False)
    psw_d = nc.declare_dram_parameter("pswap", [128, 128], bf16, isOutput=False)
    m01_d = nc.declare_dram_parameter("m01", [128, 8 * 128], bf16, isOutput=False)
    psel_d = nc.declare_dram_parameter("psel", [2, 128], bf16, isOutput=False)
    y_d = nc.declare_dram_parameter("y", [T, C], f32, isOutput=True)

    with tile.TileContext(nc) as tc:
        with (
            tc.tile_pool(name="persist", bufs=1) as persist,
            tc.tile_pool(name="tmp", bufs=4) as tmp,
            tc.tile_pool(name="ppool", bufs=5) as ppool,
            tc.tile_pool(name="outp", bufs=4) as outp,
            tc.tile_pool(name="pssc", bufs=2, space="PSUM") as pssc,
            tc.tile_pool(name="psyt", bufs=2, space="PSUM") as psyt,
        ):
            # ---- persistent SBUF residents + input DMA (qk weight cols first
            # so the first head pair can start ASAP) ----
            wq_sb = [persist.tile([128, 3 * C], bf16, tag=f"wq{i}", name=f"wq{i}") for i in range(NCT)]
            xt_sb = [persist.tile([128, T], bf16, tag=f"xt{i}", name=f"xt{i}") for i in range(NCT)]
            wp_sb = [persist.tile([128, C], bf16, tag=f"wp{i}", name=f"wp{i}") for i in range(NCT)]
            cc_sb = persist.tile([128, T], bf16, tag="cc")
            ss_sb = persist.tile([128, T], bf16, tag="ss")
            psw_sb = persist.tile([128, 128], bf16, tag="psw")
            m01_sb = persist.tile([128, 8 * 128], bf16, tag="m01")
            psel_sb = persist.tile([2, 128], bf16, tag="psel")
            # qk-projection inputs first (so the exp stream starts ASAP),
            # then rope tables, v weight cols, proj weights
            nc.sync.dma_start(psw_sb[:], psw_d[:])
            for i in range(NCT):
                r0 = i * 128
                nc.sync.dma_start(wq_sb[i][:, 0 : 2 * C], wq_d[r0 : r0 + 128, 0 : 2 * C])
                nc.sync.dma_start(xt_sb[i][:], xt_d[r0 : r0 + 128, :])
            nc.sync.dma_start(cc_sb[:], cc_d[:])
            nc.sync.dma_start(ss_sb[:], ss_d[:])
            nc.sync.dma_start(m01_sb[:], m01_d[:])
            nc.sync.dma_start(psel_sb[:], psel_d[:])

            qk_sb = [persist.tile([128, T], bf16, tag=f"qk{i}", name=f"qk{i}") for i in range(2 * NCT)]
            v_sb = [persist.tile([128, H, D + 1], bf16, tag=f"v{i}", name=f"v{i}") for i in range(NT)]
            yn_sb = [persist.tile([128, T], bf16, tag=f"yn{i}", name=f"yn{i}") for i in range(NCT)]

            # ---- emitters ----
            def emit_qk_mm(jt, old_on_act=False):
                """QKV projection matmuls for one qk row-tile + psum->sbuf
                copy (split in halves so it starts after chain 1). The copy
                goes to Act in the preamble (idle there) else DVE."""
                qkps = psyt.tile([128, T], f32, tag="yt", name="qkps")
                old = tmp.tile([128, T], bf16, tag="old", name="old", bufs=2)
                for tch in range(2):
                    t0 = tch * 512
                    for ct in range(NCT):
                        nc.tensor.matmul(
                            qkps[:, t0 : t0 + 512],
                            lhsT=wq_sb[ct][:, jt * 128 : (jt + 1) * 128],
                            rhs=xt_sb[ct][:, t0 : t0 + 512],
                            start=(ct == 0),
                            stop=(ct == NCT - 1),
                        )
                    if old_on_act:
                        nc.scalar.copy(old[:, t0 : t0 + 512], qkps[:, t0 : t0 + 512])
                    else:
                        nc.vector.tensor_copy(old[:, t0 : t0 + 512], qkps[:, t0 : t0 + 512])
                # rope partition swap via SBUF->SBUF DMA (frees the PE from
                # the pswap matmul)
                swp = tmp.tile([128, T], bf16, tag="swp", name="swp", bufs=2)
                for d0, s0 in ((0, 32), (32, 0), (64, 96), (96, 64)):
                    nc.sync.dma_start(swp[d0 : d0 + 32, :], old[s0 : s0 + 32, :])
                return old, swp

            def emit_qk_rope(jt, oldswp):
                """Rope combine for one qk tile: cc/ss muls, add into qk_sb."""
                old, swp = oldswp
                t2 = tmp.tile([128, T], bf16, tag="t2", name="t2", bufs=2)
                nc.vector.tensor_mul(t2[:], old[:], cc_sb[:])
                t1 = tmp.tile([128, T], bf16, tag="t1", name="t1", bufs=2)
                nc.vector.tensor_mul(t1[:], swp[:], ss_sb[:])
                nc.gpsimd.tensor_add(qk_sb[jt][:], t1[:], t2[:])

            def emit_qk(p):
                o1 = emit_qk_mm(p, old_on_act=True)
                o2 = emit_qk_mm(NCT + p, old_on_act=True)
                emit_qk_rope(p, o1)
                emit_qk_rope(NCT + p, o2)

            def emit_v(tt):
                for j0, jw, h0, nh in ((0, 512, 0, 8), (512, 256, 8, 4)):
                    ps = pssc.tile([128, T], f32, tag="sc", name="psv")
                    for ct in range(NCT):
                        nc.tensor.matmul(
                            ps[:, :jw],
                            lhsT=xt_sb[ct][:, tt * 128 : (tt + 1) * 128],
                            rhs=wq_sb[ct][:, 2 * C + j0 : 2 * C + j0 + jw],
                            start=(ct == 0),
                            stop=(ct == NCT - 1),
                        )
                    nc.vector.tensor_copy(
                        v_sb[tt][:, h0 : h0 + nh, 0:D],
                        ps[:, :jw].rearrange("p (h d) -> p h d", h=nh),
                    )
                nc.gpsimd.memset(v_sb[tt][:, :, D : D + 1], 1.0)

            # packed per-head p storage: s-tile i occupies cols
            # [POFF[i], POFF[i] + 1024 - 128 i)
            POFF = [0]
            for i in range(NT):
                POFF.append(POFF[-1] + T - 128 * i)
            PTOT = POFF[-1]  # 4608

            def emit_score_tiles(h, ph, lo, hi):
                """Scores + exp + diag-mask for s-tiles [lo, hi) of head h."""
                qt = qk_sb[h // 2]
                kt = qk_sb[NCT + h // 2]
                po = (h % 2) * D
                for i in range(lo, hi):
                    s0 = i * 128
                    off = POFF[i]
                    lk = kt[po : po + D, s0 : s0 + 128]
                    sc = pssc.tile([128, T], f32, tag="sc", name="sc")
                    for t0, w in _segs(i):
                        nc.tensor.matmul(
                            sc[:, t0 : t0 + w],
                            lhsT=lk,
                            rhs=qt[po : po + D, t0 : t0 + w],
                        )
                    nc.scalar.activation(
                        ph[:, off : off + T - s0], sc[:, s0:T], EXP, scale=0.125
                    )
                    nc.gpsimd.tensor_mul(
                        ph[:, off : off + 128],
                        ph[:, off : off + 128],
                        m01_sb[:, i * 128 : (i + 1) * 128],
                    )

            def emit_pv(h, ph):
                yt = psyt.tile([D + 1, T], f32, tag="yt", name="yt")
                bank_first = [True, True]
                writes = [(i, t0, w) for i in range(NT) for (t0, w) in _segs(i)]
                last_for_bank = {}
                for widx, (i, t0, w) in enumerate(writes):
                    last_for_bank[1 if t0 >= 512 else 0] = widx
                for widx, (i, t0, w) in enumerate(writes):
                    s0 = i * 128
                    off = POFF[i]
                    b = 1 if t0 >= 512 else 0
                    nc.tensor.matmul(
                        yt[:, t0 : t0 + w],
                        lhsT=v_sb[i][:, h : h + 1, :],
                        rhs=ph[:, off + t0 - s0 : off + t0 - s0 + w],
                        start=bank_first[b],
                        stop=(last_for_bank[b] == widx),
                    )
                    bank_first[b] = False
                return yt

            def emit_norm_a(p, yts):
                """Early half of pair-p norm (DVE + DMA): ytmp copies off
                psum, sums-row gather, reciprocal."""
                spair = tmp.tile([2, T], bf16, tag="spair", name="spair", bufs=2)
                ytmps = []
                for k, yt in enumerate(yts):
                    ytmp = tmp.tile([D + 1, T], bf16, tag=f"ytmp{k}", name="ytmp", bufs=2)
                    nc.vector.tensor_copy(ytmp[:], yt[:])
                    ytmps.append(ytmp)
                    nc.sync.dma_start(spair[k : k + 1, :], ytmp[D : D + 1, :])
                invb = tmp.tile([2, T], bf16, tag="invb", name="invb", bufs=2)
                with nc.allow_low_precision(reason="softmax denom recip in bf16"):
                    nc.vector.reciprocal(invb[:], spair[:])
                return ytmps, invb

            def emit_norm_b(p, ytmps, invb, pool_tag="sc"):
                """Late half (PE + DVE): paired broadcast of 1/sums, norm muls
                into yn_sb[p]."""
                bcv = (psyt if pool_tag == "yt" else pssc).tile(
                    [128, T], f32, tag=pool_tag, name="bcv"
                )
                for t0 in (0, 512):
                    nc.tensor.matmul(
                        bcv[:, t0 : t0 + 512], lhsT=psel_sb[:], rhs=invb[:, t0 : t0 + 512]
                    )
                for k in range(2):
                    ro = k * D
                    nc.vector.tensor_mul(
                        yn_sb[p][ro : ro + D, :], ytmps[k][0:D, :], bcv[ro : ro + D, :]
                    )

            # ---- main pipeline ----
            ph_tiles = {}

            def sc_part(h, lo, hi):
                if h >= H:
                    return
                if h not in ph_tiles:
                    ph_tiles[h] = ppool.tile([128, PTOT], bf16, tag="p", name="ph")
                emit_score_tiles(h, ph_tiles[h], lo, hi)

            def emit_ytmp(k, yt):
                """Copy one head's PV output off psum; returns its ytmp."""
                ytmp = tmp.tile([D + 1, T], bf16, tag=f"ytmp{k}", name="ytmp", bufs=2)
                nc.vector.tensor_copy(ytmp[:], yt[:])
                return ytmp

            def emit_recip(p, ytmps):
                spair = tmp.tile([2, T], bf16, tag="spair", name="spair", bufs=2)
                for k in range(2):
                    nc.sync.dma_start(spair[k : k + 1, :], ytmps[k][D : D + 1, :])
                invb = tmp.tile([2, T], bf16, tag="invb", name="invb", bufs=2)
                with nc.allow_low_precision(reason="softmax denom recip in bf16"):
                    nc.vector.reciprocal(invb[:], spair[:])
                return invb

            # preamble: qk pairs 0-2 + scores 0-2, v tiles woven as PE filler
            if stage >= 1:
                emit_qk(0)
                for i in range(NCT):
                    r0 = i * 128
                    nc.sync.dma_start(
                        wq_sb[i][:, 2 * C : 3 * C], wq_d[r0 : r0 + 128, 2 * C : 3 * C]
                    )
                emit_qk(1)
                for i in range(NCT):
                    nc.sync.dma_start(wp_sb[i][:], wp_d[i * 128 : (i + 1) * 128, :])
            if stage >= 2:
                sc_part(0, 0, 4)
                oq1 = emit_qk_mm(2, old_on_act=True)
                sc_part(0, 4, NT)
                emit_qk_rope(2, oq1)
                oq2 = emit_qk_mm(NCT + 2, old_on_act=True)
                emit_qk_rope(NCT + 2, oq2)
            if stage >= 1:
                emit_v(0)
                emit_v(1)
            if stage >= 2:
                sc_part(1, 0, 4)
            if stage >= 1:
                emit_v(2)
                emit_v(3)
            if stage >= 2:
                sc_part(1, 4, NT)
            if stage >= 1:
                emit_v(4)
                emit_v(5)
            if stage >= 2:
                sc_part(2, 0, 4)
            if stage >= 1:
                emit_v(6)
                emit_v(7)
            if stage >= 2:
                sc_part(2, 4, NT)
                sc_part(3, 0, NT)

            # proj helpers: partial K-chains (ct 0..4) let the tail overlap
            # the last pair's softmax/norm; finish() adds ct=5 and ships out
            def proj_start(tt, pool_tag):
                ps = (psyt if pool_tag == "yt" else pssc).tile(
                    [128, T], f32, tag=pool_tag, name="pjps"
                )
                for j0, jw in ((0, 512), (512, 256)):
                    for ct in range(NCT - 1):
                        nc.tensor.matmul(
                            ps[:, j0 : j0 + jw],
                            lhsT=yn_sb[ct][:, tt * 128 : (tt + 1) * 128],
                            rhs=wp_sb[ct][:, j0 : j0 + jw],
                            start=(ct == 0),
                            stop=False,
                        )
                return ps

            def proj_finish(tt, ps, ksplit=False):
                ct = NCT - 1
                t0 = tt * 128
                for j0, jw in ((0, 512), (512, 256)):
                    if ksplit:
                        # contract the last pair head-by-head so the first mm
                        # only waits on head 2*ct's norm mul
                        for r in (slice(0, D), slice(D, 128)):
                            nc.tensor.matmul(
                                ps[:, j0 : j0 + jw],
                                lhsT=yn_sb[ct][r, t0 : t0 + 128],
                                rhs=wp_sb[ct][r, j0 : j0 + jw],
                                start=False,
                                stop=(r.start == D),
                            )
                    else:
                        nc.tensor.matmul(
                            ps[:, j0 : j0 + jw],
                            lhsT=yn_sb[ct][:, t0 : t0 + 128],
                            rhs=wp_sb[ct][:, j0 : j0 + jw],
                            start=False,
                            stop=True,
                        )
                osb = outp.tile([128, C], f32, tag="osb")
                nc.scalar.copy(osb[:], ps[:, 0:C])
                nc.sync.dma_start(y_d[t0 : t0 + 128, :], osb[:])

            # steady-state blocks: PVs first (their exps landed 2 blocks ago),
            # then the norm chain early, scores/qk woven to keep Act fed
            if stage >= 3:
                for p in range(NP):
                    last = p == NP - 1
                    yt0 = emit_pv(2 * p, ph_tiles.pop(2 * p))
                    yp0 = emit_ytmp(0, yt0) if stage >= 4 else None
                    yt1 = emit_pv(2 * p + 1, ph_tiles.pop(2 * p + 1))
                    if stage >= 4:
                        yp1 = emit_ytmp(1, yt1)
                        invb = emit_recip(p, (yp0, yp1))
                    sc_part(2 * p + 4, 0, 4)
                    if p + 3 < NP:
                        oq1 = emit_qk_mm(p + 3)
                    if last and stage >= 5:
                        pj0 = proj_start(0, "sc")
                        pj1 = proj_start(1, "sc")
                    sc_part(2 * p + 4, 4, NT)
                    if p + 3 < NP:
                        emit_qk_rope(p + 3, oq1)
                        oq2 = emit_qk_mm(NCT + p + 3)
                    if last and stage >= 5:
                        pj2 = proj_start(2, "yt")
                    sc_part(2 * p + 5, 0, 4)
                    if stage >= 4:
                        emit_norm_b(p, (yp0, yp1), invb, "yt" if last else "sc")
                    if p + 3 < NP:
                        emit_qk_rope(NCT + p + 3, oq2)
                    sc_part(2 * p + 5, 4, NT)
                    if last and stage >= 5:
                        proj_finish(0, pj0, ksplit=True)
                        proj_finish(1, pj1)
                        proj_finish(2, pj2)

            # ---- debug probes for truncated stages ----
            if stage < 5:
                yb = y_d[:].bitcast(bf16)  # [T, 2C] bf16 view of the fp32 output
                if stage == 0:
                    nc.gpsimd.dma_start(yb[0:128, 0:T], xt_sb[0][:])
                elif stage == 1:
                    nc.gpsimd.dma_start(yb[0:128, 0:T], qk_sb[0][:])
                    nc.gpsimd.dma_start(yb[128:256, 0:T], qk_sb[6][:])
                elif stage == 2:
                    nc.gpsimd.dma_start(
                        yb[0:128, 0 : H * (D + 1)],
                        v_sb[0][:].rearrange("p h d -> p (h d)"),
                    )
                elif stage >= 4:
                    nc.gpsimd.dma_start(yb[0:128, 0:T], yn_sb[0][:])

            # ---- proj: out = yT_norm.T @ w_projT (tt 0-2 handled in-block) ----
            for tt in range(3, NT) if stage >= 5 else []:
                ps = proj_start(tt, ("sc", "yt")[tt % 2])
                proj_finish(tt, ps)

    split_multiwaits(nc)
    return nc


def _get_compiled():
    if "nc" not in _CACHE:
        _CACHE["nc"] = _build_nc()
        cc, ss, pswap, m01, psel = _host_tables()
        _CACHE["tables"] = {
            "cc": cc.astype(BF16),
            "ss": ss.astype(BF16),
            "pswap": pswap.astype(BF16),
            "m01": m01.astype(BF16),
            "psel": psel.astype(BF16),
        }
    return _CACHE["nc"], _CACHE["tables"]


def kernel(x, w_qkv, w_proj):
    from concourse.bass_utils import run_bass_kernel_spmd

    nc, tables = _get_compiled()
    x = np.asarray(x, dtype=np.float32)
    wq_t = np.ascontiguousarray(np.asarray(w_qkv, np.float32).T).astype(BF16)
    wp_t = np.ascontiguousarray(np.asarray(w_proj, np.float32).T).astype(BF16)
    in_maps = []
    for b in range(B):
        in_maps.append(
            {
                "xt": np.ascontiguousarray(x[b].T).astype(BF16),
                "wqkvt": wq_t,
                "wprojt": wp_t,
                **tables,
            }
        )
    res = run_bass_kernel_spmd(nc, in_maps, core_ids=list(range(B)))
    return np.stack([res.results[b]["y"].astype(np.float32) for b in range(B)], axis=0)
